# revision 25
# baseline (speedup 1.0000x reference)
"""Trainium2 Bass kernel for causal MHA with RoPE (nn_MHA_14164802142240).

Full-input contract: kernel(x, W_qkv, W_o) -> [B, S, E], distributed
internally across 8 NeuronCores as (batch x head-group): core c handles
batch c//4 and heads (c%4)*4 .. (c%4)*4+3.  Each core computes its 4 heads'
attention plus the partial output projection over its W_o column block; the
host sums the 4 head-group partials per batch (fp32).

v3: chunk-pipelined schedule.  The sequence is processed in 4 chunks of 512;
QKV projection / RoPE / repack for chunk c+1 are interleaved into the
attention inner loop of chunk c (finer-grained hooks per head) so the Tensor
engine never drains and Scalar (softmax exp, the co-bottleneck) starts
~20us into the kernel.  Scores use 64-row contraction (no Q/K duplication).
Inputs are host-prepacked to SBUF layouts and split into latency-sized DMAs
on three queues; the chunk-0 projection runs e-outer so matmuls chase the
DMA wavefront.  Softmax 1/l broadcast is a Tensor outer-product into PSUM
(deferred one head so the reciprocal chain hides); output is fp16, summed
on host in fp32.  Scalar runs exp only.
"""

import numpy as np

B, S, E = 2, 2048, 1024
H, D = 16, 64
HG = 4           # heads per core
NCORES = 8
SC = 512         # seq chunk
NSC = S // SC    # 4
NE = E // 128    # 8 contraction chunks
VW = 66          # per-head V stationary width: 64 v cols + 2 ones cols
LOOKP = 2        # exp->AV lookahead in tile-pairs

_COMPILED = None


def _build_bass():
    import concourse.bass as bass
    import concourse.mybir as mybir
    import concourse.tile as tile
    from concourse import bacc
    from contextlib import ExitStack

    f32 = mybir.dt.float32
    f16 = mybir.dt.float16
    Exp = mybir.ActivationFunctionType.Exp

    nc = bacc.Bacc("TRN2", target_bir_lowering=False, debug=False,
                   enable_asserts=False)

    # host-prepacked DRAM layouts (identical to the SBUF tile layouts)
    xp_d = nc.dram_tensor("xp", [128, NSC * NE * SC], f16,
                          kind="ExternalInput").ap()
    wqk_d = nc.dram_tensor("wqk", [128, NE * SC], f16,
                           kind="ExternalInput").ap()
    wv_d = nc.dram_tensor("wv", [128, NE * 256], f16,
                          kind="ExternalInput").ap()
    wo_d = nc.dram_tensor("wo", [128, 2 * E], f16, kind="ExternalInput").ap()
    # cs: per-chunk blocks [cos(512) | sin(512)] x 4 chunks
    cs_d = nc.dram_tensor("cs", [128, 2 * S], f16, kind="ExternalInput").ap()
    mask_d = nc.dram_tensor("masks", [128, 4 * SC], f16,
                            kind="ExternalInput").ap()
    out_d = nc.dram_tensor("out", [S, E], f16, kind="ExternalOutput").ap()

    with tile.TileContext(nc) as tc, ExitStack() as ctx:
        pconst = ctx.enter_context(tc.tile_pool(name="const", bufs=1))
        px = ctx.enter_context(tc.tile_pool(name="xbuf", bufs=1))
        pqk = ctx.enter_context(tc.tile_pool(name="qkbuf", bufs=1))
        pv = ctx.enter_context(tc.tile_pool(name="vbuf", bufs=1))
        py = ctx.enter_context(tc.tile_pool(name="ybuf", bufs=1))
        pob = ctx.enter_context(tc.tile_pool(name="outbuf", bufs=2))
        ptmp = ctx.enter_context(tc.tile_pool(name="ropetmp", bufs=2))
        pp = ctx.enter_context(tc.tile_pool(name="pbuf", bufs=4))
        psm = ctx.enter_context(tc.tile_pool(name="small", bufs=3))
        ps_m = ctx.enter_context(tc.tile_pool(name="ps_m", bufs=2,
                                              space="PSUM"))
        ps_s = ctx.enter_context(tc.tile_pool(name="ps_s", bufs=2,
                                              space="PSUM"))
        ps_y = ctx.enter_context(tc.tile_pool(name="ps_y", bufs=2,
                                              space="PSUM"))

        cs_t = pconst.tile([128, 2 * S], f16, name="cs_t", tag="cs")
        mask_t = pconst.tile([128, 4 * SC], f16, name="mask_t", tag="masks")
        wqk_t = pconst.tile([128, NE * SC], f16, name="wqk_t", tag="wqk")
        wv_t = pconst.tile([128, NE * 256], f16, name="wv_t", tag="wv")
        wo_t = pconst.tile([128, 2 * E], f16, name="wo_t", tag="wo")
        ones_t = pconst.tile([1, 128], f16, name="ones_t", tag="ones")
        xt = [px.tile([128, NE * SC], f16, name=f"x{c}", tag=f"x{c}")
              for c in range(NSC)]
        # q/k raw proj output: block 0 = x0 rows, block 1 = x1 rows
        qraw = pqk.tile([128, 2 * S], f16, name="qraw", tag="qraw")
        kraw = pqk.tile([128, 2 * S], f16, name="kraw", tag="kraw")
        # per-head-pair q/k: rows 0:64 = even head (x0 32 | x1 32), 64:128 odd
        qd2 = [pqk.tile([128, S], f16, name=f"qd{i}", tag=f"qd{i}")
               for i in range(2)]
        kd2 = [pqk.tile([128, S], f16, name=f"kd{i}", tag=f"kd{i}")
               for i in range(2)]
        vt = [pv.tile([128, HG * VW], f16, name=f"v{st}", tag=f"v{st}")
              for st in range(S // 128)]
        yT = [py.tile([128, S], f32, name=f"y{i}", tag=f"y{i}")
              for i in range(2)]
        yT2 = [py.tile([128, S], f16, name=f"y2{i}", tag=f"y2{i}")
               for i in range(2)]

        # ---- input loads.  Only ~7 DMA completion-semaphore slots exist
        # globally, so issue exactly 7 upfront (2-way splits of wqk and x0
        # plus the small chunk-0 constants); everything else is deferred
        # into the pipeline via load hooks.
        for lo, hi in ((0, 1), (1, 4), (4, 8)):
            nc.sync.dma_start(
                wqk_t[:, lo * SC:hi * SC], wqk_d[:, lo * SC:hi * SC])
            nc.scalar.dma_start(
                xt[0][:, lo * SC:hi * SC], xp_d[:, lo * SC:hi * SC])
        nc.gpsimd.dma_start(cs_t[:, 0:2 * SC], cs_d[:, 0:2 * SC])
        nc.gpsimd.dma_start(wv_t[:], wv_d)
        nc.gpsimd.memset(ones_t[:], 1.0)

        def load_x(c):
            nc.sync.dma_start(xt[c][:],
                              xp_d[:, c * NE * SC:(c + 1) * NE * SC])

        def load_cs(c):
            nc.gpsimd.dma_start(cs_t[:, c * 2 * SC:(c + 1) * 2 * SC],
                                cs_d[:, c * 2 * SC:(c + 1) * 2 * SC])

        def load_wo():
            nc.gpsimd.dma_start(wo_t[:], wo_d)

        ESCALE = 0.125
        dq_i = [0]

        def qkproj(c, jt):
            pq = ps_m.tile([128, SC], f32, name="pq", tag="psm")
            for e in range(NE):
                nc.tensor.matmul(
                    pq[:],
                    lhsT=wqk_t[:, e * SC + jt * 128:e * SC + (jt + 1) * 128],
                    rhs=xt[c][:, e * SC:(e + 1) * SC],
                    start=(e == 0), stop=(e == NE - 1))
            dst = (qraw if jt < 2 else kraw)
            nc.vector.tensor_copy(
                dst[:, (jt % 2) * S + c * SC:(jt % 2) * S + (c + 1) * SC],
                pq[:])

        def qkproj0():
            # chunk-0 projection, e-outer so matmuls chase the DMA
            # wavefront; rope reads the PSUM accumulators directly.
            psq = [ps_s.tile([128, 2 * SC], f32, name=f"psq{i}", tag="pss")
                   for i in range(2)]
            for e in range(NE):
                for jt in range(4):
                    nc.tensor.matmul(
                        psq[jt // 2][:, (jt % 2) * SC:(jt % 2 + 1) * SC],
                        lhsT=wqk_t[:, e * SC + jt * 128:
                                   e * SC + (jt + 1) * 128],
                        rhs=xt[0][:, e * SC:(e + 1) * SC],
                        start=(e == 0), stop=(e == NE - 1))
            return psq

        def rope0(qk, psq):
            # rope chunk 0 straight out of PSUM into qraw/kraw (no copy)
            t = qraw if qk == "q" else kraw
            p0 = psq[:, 0:SC]
            p1 = psq[:, SC:2 * SC]
            x0 = t[:, 0:SC]
            x1 = t[:, S:S + SC]
            cos_c = cs_t[:, 0:SC]
            sin_c = cs_t[:, SC:2 * SC]
            tmp = ptmp.tile([128, SC], f16, name="tmp", tag="rt0")
            tmp2 = ptmp.tile([128, SC], f16, name="tmp2", tag="rt1")
            nc.vector.tensor_mul(tmp[:], p0, sin_c)    # x0*sin
            nc.vector.tensor_mul(tmp2[:], p1, sin_c)   # x1*sin
            nc.vector.tensor_mul(x0, p0, cos_c)        # x0*cos
            nc.vector.tensor_mul(x1, p1, cos_c)        # x1*cos
            nc.vector.tensor_sub(x0, x0, tmp2[:])      # x0' = x0 c - x1 s
            nc.vector.tensor_add(x1, x1, tmp[:])       # x1' = x0 s + x1 c

        def vproj(c, h, copy_eng=None):
            st = 4 * c + h
            pv_ = ps_m.tile([128, SC], f32, name="pv_", tag="psm")
            for e in range(NE):
                nc.tensor.matmul(
                    pv_[:, 0:HG * D],
                    lhsT=xt[c][:, e * SC + h * 128:e * SC + (h + 1) * 128],
                    rhs=wv_t[:, e * 256:(e + 1) * 256],
                    start=(e == 0), stop=(e == NE - 1))
            v_view = vt[st].rearrange("p (h w) -> p h w", h=HG)
            (copy_eng or nc.vector).tensor_copy(
                v_view[:, :, 0:D],
                pv_[:, 0:HG * D].rearrange("p (h d) -> p h d", h=HG))
            nc.gpsimd.memset(v_view[:, :, D:VW], 1.0)

        def rope(c, qk):
            t = qraw if qk == "q" else kraw
            x0 = t[:, 0 * S + c * SC:0 * S + (c + 1) * SC]
            x1 = t[:, 1 * S + c * SC:1 * S + (c + 1) * SC]
            cos_c = cs_t[:, c * 2 * SC:c * 2 * SC + SC]
            sin_c = cs_t[:, c * 2 * SC + SC:(c + 1) * 2 * SC]
            tmp = ptmp.tile([128, SC], f16, name="tmp", tag="rt0")
            tmp2 = ptmp.tile([128, SC], f16, name="tmp2", tag="rt1")
            nc.vector.tensor_mul(tmp[:], x0, sin_c)    # x0*sin
            nc.vector.tensor_mul(tmp2[:], x1, sin_c)   # x1*sin
            nc.vector.tensor_mul(x0, x0, cos_c)        # x0*cos
            nc.vector.tensor_mul(x1, x1, cos_c)        # x1*cos
            nc.vector.tensor_sub(x0, x0, tmp2[:])      # x0' = x0 c - x1 s
            nc.vector.tensor_add(x1, x1, tmp[:])       # x1' = x0 s + x1 c

        def repack(c, qk, h):
            src_t = qraw if qk == "q" else kraw
            dst_t = (qd2 if qk == "q" else kd2)[h // 2]
            r0 = 64 * (h % 2)
            q = nc.sync if dq_i[0] % 2 == 0 else nc.gpsimd
            dq_i[0] += 1
            for a in range(2):
                q.dma_start(
                    dst_t[r0 + 32 * a:r0 + 32 * a + 32,
                          c * SC:(c + 1) * SC],
                    src_t[32 * h:32 * h + 32,
                          a * S + c * SC:a * S + (c + 1) * SC])

        pending_norm = []

        def norm_finish():
            while pending_norm:
                c, h, rrow = pending_norm.pop(0)
                h2, hb = h // 2, 64 * (h % 2)
                rbc = ps_m.tile([128, SC], f32, name="rbc", tag="psm")
                nc.tensor.matmul(rbc[:], lhsT=ones_t[:], rhs=rrow[:],
                                 start=True, stop=True)
                nc.vector.tensor_mul(
                    yT2[h2][hb:hb + 64, c * SC:(c + 1) * SC],
                    yT[h2][hb:hb + 64, c * SC:(c + 1) * SC],
                    rbc[hb:hb + 64, :])

        def attn_head(c, h, units=None, upp=0):
            # units: deferred fine-grained Tensor work (o-proj pieces) fed
            # in just ahead of each scores pair so the Tensor queue always
            # has exp-independent work to chew while Scalar catches up.
            norm_finish()
            nt = 4 * c + 4
            npair = nt // 2
            h2, hb = h // 2, 64 * (h % 2)
            psy = ps_y.tile([128, SC], f32, name="psy", tag="psy")
            pts = {}
            for pi in range(npair + LOOKP):
                if units:
                    for _ in range(upp):
                        if units:
                            units.pop(0)()
                if pi < npair:
                    pss = ps_s.tile([128, 2 * SC], f32, name="pss", tag="pss")
                    pt = pp.tile([128, 2 * SC], f16, name="pt", tag="pt")
                    for half in (0, 1):
                        t = 2 * pi + half
                        rg = max(0, 128 * (t - 4 * c))
                        nc.tensor.matmul(
                            pss[:, half * SC + rg:(half + 1) * SC],
                            lhsT=kd2[h2][hb:hb + 64, t * 128:(t + 1) * 128],
                            rhs=qd2[h2][hb:hb + 64,
                                        c * SC + rg:(c + 1) * SC],
                            start=True, stop=True)
                    g0 = 2 * pi - 4 * c
                    if g0 < 0:
                        nc.scalar.activation(pt[:], pss[:], Exp, scale=ESCALE)
                    else:
                        r0 = 128 * g0
                        nc.scalar.activation(
                            pt[:, r0:], pss[:, r0:], Exp, scale=ESCALE)
                        for half in (0, 1):
                            g = g0 + half
                            lo = half * SC
                            rr = 128 * g
                            nc.vector.tensor_mul(
                                pt[:, lo + rr:lo + SC],
                                pt[:, lo + rr:lo + SC],
                                mask_t[:, g * SC + rr:(g + 1) * SC])
                    pts[pi] = pt
                pp_ = pi - LOOKP
                if 0 <= pp_ < npair:
                    ptc = pts.pop(pp_)
                    for half in (0, 1):
                        t = 2 * pp_ + half
                        rg = max(0, 128 * (t - 4 * c))
                        nc.tensor.matmul(
                            psy[0:VW, rg:],
                            lhsT=vt[t][:, VW * h:VW * (h + 1)],
                            rhs=ptc[:, half * SC + rg:(half + 1) * SC],
                            start=(t == 0), stop=(t == nt - 1))
            # denominator reciprocal first (longest chain), then y copy
            lrow = psm.tile([1, SC], f32, name="lrow", tag="lrow")
            nc.vector.tensor_copy(lrow[:], psy[D:D + 1, :])
            rrow32 = psm.tile([1, SC], f32, name="rrow32", tag="rrow32")
            nc.vector.reciprocal_approx_fast(rrow32[:], lrow[:])
            rrow = psm.tile([1, SC], f16, name="rrow", tag="rrow")
            nc.vector.tensor_copy(rrow[:], rrow32[:])
            nc.vector.tensor_copy(
                yT[h2][hb:hb + 64, c * SC:(c + 1) * SC], psy[0:D, :])
            pending_norm.append((c, h, rrow))

        def oproj_units(c, final=False):
            ob = pob.tile([128, 4 * E], f16, name="ob", tag="ob")

            def unit(st, ec):
                def emit():
                    pso = ps_m.tile([128, SC], f32, name="pso", tag="psm")
                    for ft in range(2):
                        nc.tensor.matmul(
                            pso[:],
                            lhsT=yT2[ft][:, st * 128:(st + 1) * 128],
                            rhs=wo_t[:, ft * E + ec * SC:
                                     ft * E + (ec + 1) * SC],
                            start=(ft == 0), stop=(ft == 1))
                    dst = ob[:, (st % 4) * E + ec * SC:
                             (st % 4) * E + (ec + 1) * SC]
                    nc.vector.tensor_copy(dst, pso[:])
                    if final and st >= 4 * c + 2:
                        # drain the tail at ec granularity
                        q = nc.sync if dq_i[0] % 2 == 0 else nc.gpsimd
                        dq_i[0] += 1
                        q.dma_start(
                            out_d[st * 128:(st + 1) * 128,
                                  ec * SC:(ec + 1) * SC],
                            ob[:, (st % 4) * E + ec * SC:
                               (st % 4) * E + (ec + 1) * SC])
                    elif ec == 1:
                        q = nc.sync if dq_i[0] % 2 == 0 else nc.gpsimd
                        dq_i[0] += 1
                        q.dma_start(
                            out_d[st * 128:(st + 1) * 128, :],
                            ob[:, (st % 4) * E:(st % 4 + 1) * E])
                return emit

            return [unit(st, ec)
                    for st in range(4 * c, 4 * c + 4) for ec in range(2)]

        # ---- prologue: chunk 0 qk projection + rope + repack; the chunk-0
        # v projection fills the Tensor queue while repack DMAs land.
        psq = qkproj0()
        rope0("q", psq[0])
        rope0("k", psq[1])
        for h in range(4):
            repack(0, "q", h)
            repack(0, "k", h)
        nc.gpsimd.dma_start(mask_t[:], mask_d)
        for h in range(4):
            vproj(0, h)
        load_x(1)
        load_cs(1)

        # ---- main chunk pipeline; next-chunk work hooks per head:
        # h0: qk jt0,jt1 | h1: oproj(c-1), rope+repack q, qk jt2,jt3 |
        # h2: rope+repack k, v st0,st1 | h3: v st2,st3
        # oproj(c) deferred into attention of chunk c+1, head 1.
        units = []
        for c in range(NSC):
            npair_c = 4 * (2 * c + 2) + 4 * LOOKP
            upp = max(1, (len(units) + npair_c - 1) // npair_c)
            for h in range(4):
                attn_head(c, h, units=units, upp=upp)
                if c == 0 and h == 3:
                    load_x(2)
                    load_cs(2)
                    load_wo()
                if c == 1 and h == 1:
                    load_x(3)
                    load_cs(3)
                if c < NSC - 1:
                    if h == 0:
                        qkproj(c + 1, 0)
                        qkproj(c + 1, 1)
                        rope(c + 1, "q")
                    elif h == 1:
                        for hh in range(4):
                            repack(c + 1, "q", hh)
                        qkproj(c + 1, 2)
                        qkproj(c + 1, 3)
                        rope(c + 1, "k")
                    elif h == 2:
                        for hh in range(4):
                            repack(c + 1, "k", hh)
                        vproj(c + 1, 0)
                        vproj(c + 1, 1)
                    else:
                        vproj(c + 1, 2)
                        vproj(c + 1, 3)
            for u in units:
                u()
            units = oproj_units(c, final=(c == NSC - 1))
        norm_finish()
        for u in units:
            u()

    nc.compile()
    return nc


def _host_inputs(x, W_qkv, W_o):
    """Build the 8 per-core input maps (fp16, SBUF-layout prepacked)."""
    thetas = 10000.0 ** (-2.0 * (np.arange(D // 2, dtype=np.float32) / D))
    freqs = np.arange(S, dtype=np.float32)[:, None] * thetas[None, :]  # [S,32]
    cosT = np.tile(np.cos(freqs).astype(np.float16).T, (4, 1))  # [128, S]
    sinT = np.tile(np.sin(freqs).astype(np.float16).T, (4, 1))
    # per-chunk blocks [cos_c | sin_c]
    cs = np.ascontiguousarray(np.concatenate(
        [np.concatenate([cosT[:, c * SC:(c + 1) * SC],
                         sinT[:, c * SC:(c + 1) * SC]], axis=1)
         for c in range(NSC)], axis=1))  # [128, 2S]

    jj = np.arange(128)[:, None]
    masks = np.ascontiguousarray(np.concatenate(
        [(128 * g + jj <= np.arange(SC)[None, :]) for g in range(4)],
        axis=1).astype(np.float16))  # [128, 4*SC]

    def pack_w(wt, blk):
        # [E, cols] -> [128, NE*blk] with e-chunk b at cols b*blk:(b+1)*blk
        return np.ascontiguousarray(
            wt.reshape(NE, 128, blk).transpose(1, 0, 2).reshape(128, NE * blk))

    xps = []
    for b in range(B):
        xT = x[b].T.astype(np.float16)  # [E, S]
        chunks = [pack_w(np.ascontiguousarray(xT[:, c * SC:(c + 1) * SC]), SC)
                  for c in range(NSC)]
        xps.append(np.ascontiguousarray(np.concatenate(chunks, axis=1)))

    in_maps = []
    for core in range(NCORES):
        b, hg = core // 4, core % 4
        heads = range(hg * HG, (hg + 1) * HG)
        qx0 = [h * D + 2 * m for h in heads for m in range(D // 2)]
        qx1 = [h * D + 2 * m + 1 for h in heads for m in range(D // 2)]
        rows = (qx0 + qx1 + [E + i for i in qx0] + [E + i for i in qx1])
        wqk = pack_w(W_qkv[rows].T.astype(np.float16), SC)  # [128, 4096]
        vrows = [2 * E + h * D + d for h in heads for d in range(D)]
        wv = pack_w(W_qkv[vrows].T.astype(np.float16), 256)  # [128, 2048]
        wo_sl = W_o[:, hg * HG * D:(hg + 1) * HG * D].T.astype(np.float16)
        wo = np.ascontiguousarray(
            wo_sl.reshape(2, 128, E).transpose(1, 0, 2).reshape(128, 2 * E))
        in_maps.append({
            "xp": xps[b], "wqk": wqk, "wv": wv, "wo": wo,
            "cs": cs, "masks": masks,
        })
    return in_maps


def kernel(x, W_qkv, W_o):
    global _COMPILED
    x = np.ascontiguousarray(np.asarray(x, dtype=np.float32))
    W_qkv = np.ascontiguousarray(np.asarray(W_qkv, dtype=np.float32))
    W_o = np.ascontiguousarray(np.asarray(W_o, dtype=np.float32))

    if _COMPILED is None:
        _COMPILED = _build_bass()
    nc = _COMPILED

    from concourse.bass_utils import run_bass_kernel_spmd
    in_maps = _host_inputs(x, W_qkv, W_o)
    res = run_bass_kernel_spmd(nc, in_maps, core_ids=list(range(NCORES)))
    out = np.zeros((B, S, E), dtype=np.float32)
    for core in range(NCORES):
        out[core // 4] += res.results[core]["out"].astype(np.float32)
    return out


# revision 26
# speedup vs baseline: 1.2559x; 1.2559x over previous
"""Trainium2 Bass kernel for causal MHA with RoPE (nn_MHA_14164802142240).

Full-input contract: kernel(x, W_qkv, W_o) -> [B, S, E], distributed
internally across 8 NeuronCores as (batch x head-group): core c handles
batch c//4 and heads (c%4)*4 .. (c%4)*4+3.  Each core computes its 4 heads'
attention plus the partial output projection over its W_o column block; the
host sums the 4 head-group partials per batch (fp32).

Chunk-pipelined schedule: the sequence is processed in 4 chunks of 512;
QKV projection / RoPE / repack for chunk c+1 are interleaved into the
attention inner loop of chunk c so the Tensor engine never drains and the
Scalar engine (softmax exp, the co-bottleneck) starts early.  Scores use
64-row contraction (no Q/K row duplication).  Inputs are host-prepacked to
SBUF layouts and split into latency-sized DMAs; the chunk-0 projection runs
e-outer so matmuls chase the DMA wavefront.  Softmax 1/l broadcast is a
Tensor outer-product into PSUM (deferred one head so the reciprocal chain
hides); output is fp16, summed on host in fp32.  Scalar runs exp only.
"""

import numpy as np

B, S, E = 2, 2048, 1024
H, D = 16, 64
HG = 4           # heads per core
NCORES = 8
SC = 512         # seq chunk
NSC = S // SC    # 4
NE = E // 128    # 8 contraction chunks
VW = 66          # per-head V stationary width: 64 v cols + 2 ones cols
LOOKP = 2        # exp->AV lookahead in tile-pairs

_COMPILED = None


def _build_bass():
    import concourse.bass as bass
    import concourse.mybir as mybir
    import concourse.tile as tile
    from concourse import bacc
    from contextlib import ExitStack

    f32 = mybir.dt.float32
    f16 = mybir.dt.float16
    Exp = mybir.ActivationFunctionType.Exp

    nc = bacc.Bacc("TRN2", target_bir_lowering=False, debug=False,
                   enable_asserts=False)

    # host-prepacked DRAM layouts (identical to the SBUF tile layouts)
    xp_d = nc.dram_tensor("xp", [128, NSC * NE * SC], f16,
                          kind="ExternalInput").ap()
    wqk_d = nc.dram_tensor("wqk", [128, NE * SC], f16,
                           kind="ExternalInput").ap()
    wv_d = nc.dram_tensor("wv", [128, NE * 256], f16,
                          kind="ExternalInput").ap()
    wo_d = nc.dram_tensor("wo", [128, 2 * E], f16, kind="ExternalInput").ap()
    # cs: per-chunk blocks [cos(512) | sin(512)] x 4 chunks
    cs_d = nc.dram_tensor("cs", [128, 2 * S], f16, kind="ExternalInput").ap()
    mask_d = nc.dram_tensor("masks", [128, 4 * SC], f16,
                            kind="ExternalInput").ap()
    out_d = nc.dram_tensor("out", [S, E], f16, kind="ExternalOutput").ap()

    with tile.TileContext(nc) as tc, ExitStack() as ctx:
        pconst = ctx.enter_context(tc.tile_pool(name="const", bufs=1))
        px = ctx.enter_context(tc.tile_pool(name="xbuf", bufs=1))
        pqk = ctx.enter_context(tc.tile_pool(name="qkbuf", bufs=1))
        pv = ctx.enter_context(tc.tile_pool(name="vbuf", bufs=1))
        py = ctx.enter_context(tc.tile_pool(name="ybuf", bufs=1))
        pob = ctx.enter_context(tc.tile_pool(name="outbuf", bufs=2))
        ptmp = ctx.enter_context(tc.tile_pool(name="ropetmp", bufs=2))
        pp = ctx.enter_context(tc.tile_pool(name="pbuf", bufs=4))
        psm = ctx.enter_context(tc.tile_pool(name="small", bufs=3))
        ps_m = ctx.enter_context(tc.tile_pool(name="ps_m", bufs=2,
                                              space="PSUM"))
        ps_s = ctx.enter_context(tc.tile_pool(name="ps_s", bufs=2,
                                              space="PSUM"))
        ps_y = ctx.enter_context(tc.tile_pool(name="ps_y", bufs=2,
                                              space="PSUM"))

        cs_t = pconst.tile([128, 2 * S], f16, name="cs_t", tag="cs")
        mask_t = pconst.tile([128, 4 * SC], f16, name="mask_t", tag="masks")
        wqk_t = pconst.tile([128, NE * SC], f16, name="wqk_t", tag="wqk")
        wv_t = pconst.tile([128, NE * 256], f16, name="wv_t", tag="wv")
        wo_t = pconst.tile([128, 2 * E], f16, name="wo_t", tag="wo")
        ones_t = pconst.tile([1, 128], f16, name="ones_t", tag="ones")
        xt = [px.tile([128, NE * SC], f16, name=f"x{c}", tag=f"x{c}")
              for c in range(NSC)]
        # q/k raw proj output: block 0 = x0 rows, block 1 = x1 rows
        qraw = pqk.tile([128, 2 * S], f16, name="qraw", tag="qraw")
        kraw = pqk.tile([128, 2 * S], f16, name="kraw", tag="kraw")
        # per-head-pair q/k: rows 0:64 = even head (x0 32 | x1 32), 64:128 odd
        qd2 = [pqk.tile([128, S], f16, name=f"qd{i}", tag=f"qd{i}")
               for i in range(2)]
        kd2 = [pqk.tile([128, S], f16, name=f"kd{i}", tag=f"kd{i}")
               for i in range(2)]
        vt = [pv.tile([128, HG * VW], f16, name=f"v{st}", tag=f"v{st}")
              for st in range(S // 128)]
        yT = [py.tile([128, S], f32, name=f"y{i}", tag=f"y{i}")
              for i in range(2)]
        yT2 = [py.tile([128, S], f16, name=f"y2{i}", tag=f"y2{i}")
               for i in range(2)]

        # ---- input loads
        for i in range(4):
            nc.sync.dma_start(
                wqk_t[:, i * 2 * SC:(i + 1) * 2 * SC],
                wqk_d[:, i * 2 * SC:(i + 1) * 2 * SC])
            nc.scalar.dma_start(
                xt[0][:, i * 2 * SC:(i + 1) * 2 * SC],
                xp_d[:, i * 2 * SC:(i + 1) * 2 * SC])
        nc.gpsimd.dma_start(cs_t[:, 0:2 * SC], cs_d[:, 0:2 * SC])
        nc.gpsimd.dma_start(wv_t[:], wv_d)
        nc.gpsimd.memset(ones_t[:], 1.0)
        nc.gpsimd.dma_start(mask_t[:], mask_d)
        for c in range(1, NSC):
            nc.scalar.dma_start(cs_t[:, c * 2 * SC:(c + 1) * 2 * SC],
                                cs_d[:, c * 2 * SC:(c + 1) * 2 * SC])
        for c in range(1, NSC):
            nc.scalar.dma_start(xt[c][:],
                                xp_d[:, c * NE * SC:(c + 1) * NE * SC])
        nc.scalar.dma_start(wo_t[:], wo_d)

        ESCALE = 0.125
        dq_i = [0]

        def qkproj(c, jt):
            pq = ps_m.tile([128, SC], f32, name="pq", tag="psm")
            for e in range(NE):
                nc.tensor.matmul(
                    pq[:],
                    lhsT=wqk_t[:, e * SC + jt * 128:e * SC + (jt + 1) * 128],
                    rhs=xt[c][:, e * SC:(e + 1) * SC],
                    start=(e == 0), stop=(e == NE - 1))
            dst = (qraw if jt < 2 else kraw)
            nc.vector.tensor_copy(
                dst[:, (jt % 2) * S + c * SC:(jt % 2) * S + (c + 1) * SC],
                pq[:])

        def qkproj0():
            # chunk-0 projection, e-outer so matmuls chase the DMA wavefront
            psq = [ps_s.tile([128, 2 * SC], f32, name=f"psq{i}", tag="pss")
                   for i in range(2)]
            for e in range(NE):
                for jt in range(4):
                    nc.tensor.matmul(
                        psq[jt // 2][:, (jt % 2) * SC:(jt % 2 + 1) * SC],
                        lhsT=wqk_t[:, e * SC + jt * 128:
                                   e * SC + (jt + 1) * 128],
                        rhs=xt[0][:, e * SC:(e + 1) * SC],
                        start=(e == 0), stop=(e == NE - 1))
            for jt in range(4):
                dst = (qraw if jt < 2 else kraw)
                nc.vector.tensor_copy(
                    dst[:, (jt % 2) * S:(jt % 2) * S + SC],
                    psq[jt // 2][:, (jt % 2) * SC:(jt % 2 + 1) * SC])

        def vproj(c, h):
            st = 4 * c + h
            pv_ = ps_m.tile([128, SC], f32, name="pv_", tag="psm")
            for e in range(NE):
                nc.tensor.matmul(
                    pv_[:, 0:HG * D],
                    lhsT=xt[c][:, e * SC + h * 128:e * SC + (h + 1) * 128],
                    rhs=wv_t[:, e * 256:(e + 1) * 256],
                    start=(e == 0), stop=(e == NE - 1))
            v_view = vt[st].rearrange("p (h w) -> p h w", h=HG)
            nc.vector.tensor_copy(
                v_view[:, :, 0:D],
                pv_[:, 0:HG * D].rearrange("p (h d) -> p h d", h=HG))
            nc.gpsimd.memset(v_view[:, :, D:VW], 1.0)

        def rope(c, qk):
            t = qraw if qk == "q" else kraw
            x0 = t[:, 0 * S + c * SC:0 * S + (c + 1) * SC]
            x1 = t[:, 1 * S + c * SC:1 * S + (c + 1) * SC]
            cos_c = cs_t[:, c * 2 * SC:c * 2 * SC + SC]
            sin_c = cs_t[:, c * 2 * SC + SC:(c + 1) * 2 * SC]
            tmp = ptmp.tile([128, SC], f16, name="tmp", tag="rt0")
            tmp2 = ptmp.tile([128, SC], f16, name="tmp2", tag="rt1")
            nc.vector.tensor_mul(tmp[:], x0, sin_c)    # x0*sin
            nc.vector.tensor_mul(tmp2[:], x1, sin_c)   # x1*sin
            nc.vector.tensor_mul(x0, x0, cos_c)        # x0*cos
            nc.vector.tensor_mul(x1, x1, cos_c)        # x1*cos
            nc.vector.tensor_sub(x0, x0, tmp2[:])      # x0' = x0 c - x1 s
            nc.vector.tensor_add(x1, x1, tmp[:])       # x1' = x0 s + x1 c

        def repack(c, qk, h):
            src_t = qraw if qk == "q" else kraw
            dst_t = (qd2 if qk == "q" else kd2)[h // 2]
            r0 = 64 * (h % 2)
            q = nc.sync if dq_i[0] % 2 == 0 else nc.gpsimd
            dq_i[0] += 1
            for a in range(2):
                q.dma_start(
                    dst_t[r0 + 32 * a:r0 + 32 * a + 32,
                          c * SC:(c + 1) * SC],
                    src_t[32 * h:32 * h + 32,
                          a * S + c * SC:a * S + (c + 1) * SC])

        pending_norm = []

        def norm_finish():
            while pending_norm:
                c, h, rrow = pending_norm.pop(0)
                h2, hb = h // 2, 64 * (h % 2)
                rbc = ps_m.tile([128, SC], f32, name="rbc", tag="psm")
                nc.tensor.matmul(rbc[:], lhsT=ones_t[:], rhs=rrow[:],
                                 start=True, stop=True)
                nc.vector.tensor_mul(
                    yT2[h2][hb:hb + 64, c * SC:(c + 1) * SC],
                    yT[h2][hb:hb + 64, c * SC:(c + 1) * SC],
                    rbc[hb:hb + 64, :])

        def attn_head(c, h):
            norm_finish()
            nt = 4 * c + 4
            npair = nt // 2
            h2, hb = h // 2, 64 * (h % 2)
            psy = ps_y.tile([128, SC], f32, name="psy", tag="psy")
            pts = {}
            for pi in range(npair + LOOKP):
                if pi < npair:
                    pss = ps_s.tile([128, 2 * SC], f32, name="pss", tag="pss")
                    pt = pp.tile([128, 2 * SC], f16, name="pt", tag="pt")
                    for half in (0, 1):
                        t = 2 * pi + half
                        rg = max(0, 128 * (t - 4 * c))
                        nc.tensor.matmul(
                            pss[:, half * SC + rg:(half + 1) * SC],
                            lhsT=kd2[h2][hb:hb + 64, t * 128:(t + 1) * 128],
                            rhs=qd2[h2][hb:hb + 64,
                                        c * SC + rg:(c + 1) * SC],
                            start=True, stop=True)
                    g0 = 2 * pi - 4 * c
                    if g0 < 0:
                        nc.scalar.activation(pt[:], pss[:], Exp, scale=ESCALE)
                    else:
                        r0 = 128 * g0
                        nc.scalar.activation(
                            pt[:, r0:], pss[:, r0:], Exp, scale=ESCALE)
                        for half in (0, 1):
                            g = g0 + half
                            lo = half * SC
                            rr = 128 * g
                            nc.vector.tensor_mul(
                                pt[:, lo + rr:lo + SC],
                                pt[:, lo + rr:lo + SC],
                                mask_t[:, g * SC + rr:(g + 1) * SC])
                    pts[pi] = pt
                pp_ = pi - LOOKP
                if 0 <= pp_ < npair:
                    ptc = pts.pop(pp_)
                    for half in (0, 1):
                        t = 2 * pp_ + half
                        rg = max(0, 128 * (t - 4 * c))
                        nc.tensor.matmul(
                            psy[0:VW, rg:],
                            lhsT=vt[t][:, VW * h:VW * (h + 1)],
                            rhs=ptc[:, half * SC + rg:(half + 1) * SC],
                            start=(t == 0), stop=(t == nt - 1))
            # denominator reciprocal first (longest chain), then y copy
            lrow = psm.tile([1, SC], f32, name="lrow", tag="lrow")
            nc.vector.tensor_copy(lrow[:], psy[D:D + 1, :])
            rrow32 = psm.tile([1, SC], f32, name="rrow32", tag="rrow32")
            nc.vector.reciprocal_approx_fast(rrow32[:], lrow[:])
            rrow = psm.tile([1, SC], f16, name="rrow", tag="rrow")
            nc.vector.tensor_copy(rrow[:], rrow32[:])
            nc.vector.tensor_copy(
                yT[h2][hb:hb + 64, c * SC:(c + 1) * SC], psy[0:D, :])
            pending_norm.append((c, h, rrow))

        def oproj(c):
            ob = pob.tile([128, 4 * E], f16, name="ob", tag="ob")
            for st in range(4 * c, 4 * c + 4):
                for ec in range(2):
                    pso = ps_m.tile([128, SC], f32, name="pso", tag="psm")
                    for ft in range(2):
                        nc.tensor.matmul(
                            pso[:],
                            lhsT=yT2[ft][:, st * 128:(st + 1) * 128],
                            rhs=wo_t[:, ft * E + ec * SC:
                                     ft * E + (ec + 1) * SC],
                            start=(ft == 0), stop=(ft == 1))
                    dst = ob[:, (st % 4) * E + ec * SC:
                             (st % 4) * E + (ec + 1) * SC]
                    if ec == 0:
                        nc.scalar.copy(dst, pso[:])
                    else:
                        nc.vector.tensor_copy(dst, pso[:])
                q = nc.sync if dq_i[0] % 2 == 0 else nc.gpsimd
                dq_i[0] += 1
                q.dma_start(
                    out_d[st * 128:(st + 1) * 128, :],
                    ob[:, (st % 4) * E:(st % 4 + 1) * E])

        # ---- prologue: chunk 0 projection + rope + repack
        qkproj0()
        for h in range(4):
            vproj(0, h)
        rope(0, "q")
        rope(0, "k")
        for h in range(4):
            repack(0, "q", h)
            repack(0, "k", h)

        # ---- main chunk pipeline; next-chunk work hooks per head:
        # h0: qk jt0,jt1 | h1: rope+repack q, qk jt2,jt3 |
        # h2: rope+repack k, v st0,st1 | h3: v st2,st3
        # oproj(c) deferred into attention of chunk c+1.
        for c in range(NSC):
            for h in range(4):
                attn_head(c, h)
                if h == 0 and c > 0:
                    norm_finish()
                    oproj(c - 1)
                if c < NSC - 1:
                    if h == 0:
                        qkproj(c + 1, 0)
                        qkproj(c + 1, 1)
                    elif h == 1:
                        rope(c + 1, "q")
                        for hh in range(4):
                            repack(c + 1, "q", hh)
                        qkproj(c + 1, 2)
                        qkproj(c + 1, 3)
                    elif h == 2:
                        rope(c + 1, "k")
                        for hh in range(4):
                            repack(c + 1, "k", hh)
                        vproj(c + 1, 0)
                        vproj(c + 1, 1)
                    else:
                        vproj(c + 1, 2)
                        vproj(c + 1, 3)
        norm_finish()
        oproj(NSC - 1)

    nc.compile()
    return nc


def _host_inputs(x, W_qkv, W_o):
    """Build the 8 per-core input maps (fp16, SBUF-layout prepacked)."""
    thetas = 10000.0 ** (-2.0 * (np.arange(D // 2, dtype=np.float32) / D))
    freqs = np.arange(S, dtype=np.float32)[:, None] * thetas[None, :]  # [S,32]
    cosT = np.tile(np.cos(freqs).astype(np.float16).T, (4, 1))  # [128, S]
    sinT = np.tile(np.sin(freqs).astype(np.float16).T, (4, 1))
    # per-chunk blocks [cos_c | sin_c]
    cs = np.ascontiguousarray(np.concatenate(
        [np.concatenate([cosT[:, c * SC:(c + 1) * SC],
                         sinT[:, c * SC:(c + 1) * SC]], axis=1)
         for c in range(NSC)], axis=1))  # [128, 2S]

    jj = np.arange(128)[:, None]
    masks = np.ascontiguousarray(np.concatenate(
        [(128 * g + jj <= np.arange(SC)[None, :]) for g in range(4)],
        axis=1).astype(np.float16))  # [128, 4*SC]

    def pack_w(wt, blk):
        # [E, cols] -> [128, NE*blk] with e-chunk b at cols b*blk:(b+1)*blk
        return np.ascontiguousarray(
            wt.reshape(NE, 128, blk).transpose(1, 0, 2).reshape(128, NE * blk))

    xps = []
    for b in range(B):
        xT = x[b].T.astype(np.float16)  # [E, S]
        chunks = [pack_w(np.ascontiguousarray(xT[:, c * SC:(c + 1) * SC]), SC)
                  for c in range(NSC)]
        xps.append(np.ascontiguousarray(np.concatenate(chunks, axis=1)))

    in_maps = []
    for core in range(NCORES):
        b, hg = core // 4, core % 4
        heads = range(hg * HG, (hg + 1) * HG)
        qx0 = [h * D + 2 * m for h in heads for m in range(D // 2)]
        qx1 = [h * D + 2 * m + 1 for h in heads for m in range(D // 2)]
        rows = (qx0 + qx1 + [E + i for i in qx0] + [E + i for i in qx1])
        wqk = pack_w(W_qkv[rows].T.astype(np.float16), SC)  # [128, 4096]
        vrows = [2 * E + h * D + d for h in heads for d in range(D)]
        wv = pack_w(W_qkv[vrows].T.astype(np.float16), 256)  # [128, 2048]
        wo_sl = W_o[:, hg * HG * D:(hg + 1) * HG * D].T.astype(np.float16)
        wo = np.ascontiguousarray(
            wo_sl.reshape(2, 128, E).transpose(1, 0, 2).reshape(128, 2 * E))
        in_maps.append({
            "xp": xps[b], "wqk": wqk, "wv": wv, "wo": wo,
            "cs": cs, "masks": masks,
        })
    return in_maps


def kernel(x, W_qkv, W_o):
    global _COMPILED
    x = np.ascontiguousarray(np.asarray(x, dtype=np.float32))
    W_qkv = np.ascontiguousarray(np.asarray(W_qkv, dtype=np.float32))
    W_o = np.ascontiguousarray(np.asarray(W_o, dtype=np.float32))

    if _COMPILED is None:
        _COMPILED = _build_bass()
    nc = _COMPILED

    from concourse.bass_utils import run_bass_kernel_spmd
    in_maps = _host_inputs(x, W_qkv, W_o)
    res = run_bass_kernel_spmd(nc, in_maps, core_ids=list(range(NCORES)))
    out = np.zeros((B, S, E), dtype=np.float32)
    for core in range(NCORES):
        out[core // 4] += res.results[core]["out"].astype(np.float32)
    return out
